# revision 23
# baseline (speedup 1.0000x reference)
"""ECC paged-attention kernel for 8x TRN2 NeuronCores (v2).

Semantics (from the reference): the Hamming(8,4) encode/decode round-trip is
exact and the block-table scatter/gather is the identity for any permutation
table (the graded table is arange), so the computation reduces to:

  k_d = round(k/scale_k)*scale_k   (int4 symmetric quant-dequant, per (s, kvh))
  v_d = round(v/scale_v)*scale_v
  out = causal-GQA-attention(q, k_d, v_d)   (queries = last 16 of 4096 positions)

Sharding: batch (8 sequences) across the 8 cores, pure SPMD, no collectives.

v2 changes vs the 163.6us baseline (cost model TimelineSim):
  - pass1 rounding runs in fp16 (t = x*(1/scale) + 1536; fp16 RNE rounds to
    the integer grid exactly like the fp32 + 1.5*2^23 trick, at half the
    dtype width). pass2 (t - 1536 -> bf16) is then an all-16-bit DVE op in
    4x mode, and the k transpose runs at 1 PE cycle/row instead of 2.
  - pass1 slices run mostly on GPSIMD (12), 2 on DVE (2x_2p mode), 2 on ACT;
    the DVE absmax reduce (the old bottleneck's biggest item, no fast mode
    exists for TensorReduce) is issued as ONE batched op over k and v.
  - scale_k is folded into per-head Exp activations (scale AP), removing the
    full-width scores*scale_k DVE multiply.
  - attn is stored [s, qg, kvh] so the scale_v broadcast multiply is an
    all-bf16 packed tensor_tensor (2x_1p); matmuls read strided views.

Engine budget per 128-token block (ns, cost model): DVE ~3.4k (absmax 2194,
pass2-v 327, attn*scale_v 327, smalls ~330, 2 pass1 slices), ACT ~3.5k
(8 fused Exp 1907, ktT -1536 copy 1038, 2 pass1), Pool ~3.3k (12 pass1),
PE ~1.3k, DMA 2912 (the 94.8us HBM floor).
"""

import numpy as np

B, Q, S, H, KVH, D = 8, 16, 4096, 32, 8, 128
G = H // KVH          # 4
QG = Q * G            # 64 rows per kv head
P = 128               # partitions / block size in s
NBLK = S // P         # 32
N_CORES = 8

C16 = 1536.0          # 1.5 * 2**10: fp16 add => round-to-nearest-even integer
INV7 = 1.0 / 7.0
EPS = 1e-8
INV_SQRT_D = 1.0 / float(np.sqrt(D))

_CACHE = {}


def _view(ap, dims):
    """Re-dim a tile AP: keep partition dim, replace free dims with
    [stride, count] pairs."""
    import concourse.bass as bass

    return bass.AP(tensor=ap.tensor, offset=ap.offset, ap=[ap.ap[0]] + dims)


def _build_nc(reps=1, legalize=True):
    from contextlib import ExitStack

    import concourse.bass as bass
    import concourse.tile as tile
    from concourse import mybir
    from concourse.masks import make_identity

    f32 = mybir.dt.float32
    f16 = mybir.dt.float16
    bf16 = mybir.dt.bfloat16
    AL = mybir.AluOpType
    AF = mybir.ActivationFunctionType
    AX = mybir.AxisListType

    nc = bass.Bass("TRN2", target_bir_lowering=False, debug=False,
                   num_devices=N_CORES)

    qt_d = nc.dram_tensor("qT", [D, KVH, QG], f32, kind="ExternalInput").ap()
    kv_d = nc.dram_tensor("kv", [S, 2, KVH, D], f32, kind="ExternalInput").ap()
    o_d = nc.dram_tensor("out", [Q, H, D], f32, kind="ExternalOutput").ap()
    m_d = nc.dram_tensor("maskc", [P, QG], f32, kind="ExternalInput").ap()

    with ExitStack() as ctx:
        tc = ctx.enter_context(tile.TileContext(nc))

        import os
        BUFS = dict(io=6, work=2, stat=5, fin=4, ps_kt=2, ps_sc=2, ps_fin=2, look=3)
        for kv_ in os.environ.get("KBUFS", "").split(","):
            if kv_:
                kk, vv = kv_.split("=")
                BUFS[kk] = int(vv)
        consts = ctx.enter_context(tc.tile_pool(name="consts", bufs=1))
        io = ctx.enter_context(tc.tile_pool(name="io", bufs=BUFS["io"]))
        work = ctx.enter_context(tc.tile_pool(name="work", bufs=BUFS["work"]))
        stat = ctx.enter_context(tc.tile_pool(name="stat", bufs=BUFS["stat"]))
        fin = ctx.enter_context(tc.tile_pool(name="fin", bufs=BUFS["fin"]))
        ps_kt = ctx.enter_context(tc.tile_pool(name="ps_kt", bufs=BUFS["ps_kt"], space="PSUM"))
        ps_sc = ctx.enter_context(tc.tile_pool(name="ps_sc", bufs=BUFS["ps_sc"], space="PSUM"))
        ps_acc = ctx.enter_context(tc.tile_pool(name="ps_acc", bufs=1, space="PSUM"))
        ps_fin = ctx.enter_context(tc.tile_pool(name="ps_fin", bufs=BUFS["ps_fin"], space="PSUM"))

        # ---- early loads: first kv blocks go ahead of everything so the
        # in-order HWDGE queue doesn't stall block 0 behind prologue DMAs
        LOOK = BUFS.get("look", 3)
        kv_tiles = {}

        def load_kv(blk):
            """One DMA per block: kv interleaved [P, 2(kv), KVH, D]."""
            s0 = blk * P
            kvt = io.tile([P, 2, KVH, D], f32, tag="kv")
            nc.sync.dma_start(out=kvt, in_=kv_d[s0:s0 + P])
            kv_tiles[blk] = kvt

        for b0 in range(LOOK):
            load_kv(b0)

        # ---- constants -------------------------------------------------
        ident_f16 = consts.tile([P, P], f16, tag="ident_f16")
        make_identity(nc, ident_f16)
        ident_f32 = consts.tile([P, P], f32, tag="ident_f32")
        make_identity(nc, ident_f32)
        ident_bf = consts.tile([P, P], bf16, tag="ident_bf")
        make_identity(nc, ident_bf)

        # causal-mask bias for the last s-block, [s_in_blk, qg]:
        maskb = consts.tile([P, QG], f32, tag="maskb")
        nc.sync.dma_start(out=maskb, in_=m_d)

        # ---- q prep: host-prepacked q^T, one DMA + one bf16 copy -------
        qtf = consts.tile([D, KVH, QG], f32, tag="qtf")
        nc.sync.dma_start(out=qtf, in_=qt_d)
        qTall = consts.tile([D, KVH, QG], bf16, tag="qTall")
        nc.vector.tensor_copy(qTall, qtf)
        qTs = [qTall[:, h] for h in range(KVH)]

        for _rep in range(reps):
            # ---- persistent accumulators -------------------------------
            av_ps = ps_acc.tile([D, KVH, QG], f32, tag="av")        # attn @ v (^T)
            sums_ps = ps_acc.tile([KVH, KVH, QG], f32, tag="sums")  # denom (diag)
            nc.vector.memset(av_ps, 0.0)
            nc.vector.memset(sums_ps, 0.0)

            # ---- main loop over 128-token blocks -----------------------
            for blk in range(NBLK):
                last = blk == NBLK - 1
                s0 = blk * P

                if blk + LOOK < NBLK:
                    load_kv(blk + LOOK)
                kv = kv_tiles.pop(blk)

                # batched abs-max over D -> [P, 2, KVH] (ONE DVE op)
                am = stat.tile([P, 2, KVH], f32, tag="am")
                nc.vector.tensor_reduce(am, kv, axis=AX.X, op=AL.max,
                                        apply_absolute_value=True)
                sc = stat.tile([P, 2, KVH], f32, tag="sc")
                nc.vector.tensor_scalar(sc, am, INV7, EPS, op0=AL.mult, op1=AL.max)
                rc = stat.tile([P, 2, KVH], f32, tag="rc")
                nc.vector.reciprocal(rc, sc)
                psck = stat.tile([P, KVH], f32, tag="psck")
                nc.vector.tensor_scalar(psck, sc[:, 0], INV_SQRT_D, None,
                                        op0=AL.mult)
                svb = stat.tile([P, KVH], bf16, tag="svb")
                nc.gpsimd.tensor_copy(svb, sc[:, 1])
                rvb = stat.tile([P, KVH], bf16, tag="rvb")
                nc.gpsimd.tensor_copy(rvb, rc[:, 1])

                # pass1: t = x*(1/scale) + 1536 -> fp16 (RNE to integer grid)
                t = work.tile([P, 2, KVH, D], f16, tag="t")
                for c, h in [(0, 0), (0, 1)]:
                    nc.vector.tensor_scalar(t[:, c, h], kv[:, c, h],
                                            rc[:, c, h:h + 1], C16,
                                            op0=AL.mult, op1=AL.add)
                for c, h in [(0, 2), (0, 3)]:
                    nc.scalar.activation(t[:, c, h], kv[:, c, h],
                                         AF.Copy, bias=C16, scale=rc[:, c, h:h + 1])
                for c, h in ([(0, h) for h in range(4, KVH)]
                             + [(1, h) for h in range(KVH)]):
                    nc.gpsimd.tensor_scalar(t[:, c, h], kv[:, c, h],
                                            rc[:, c, h:h + 1], C16,
                                            op0=AL.mult, op1=AL.add)

                # pass2 (v only): nib_v = t_v - 1536 -> bf16 (4x DVE)
                nv = work.tile([P, KVH, D], bf16, tag="nv")
                nc.vector.tensor_scalar(nv, t[:, 1], -C16, None, op0=AL.add)

                # k: transpose t_k (fp16, 1 cyc/row) on PE into ONE psum tile,
                # then one fused ACT pass does (t^T - 1536) -> bf16
                ktp = ps_kt.tile([D, KVH, P], f16, tag="ktp")
                for h in range(KVH):
                    nc.tensor.transpose(ktp[:, h], t[:, 0, h], ident_f16)
                ktT = work.tile([D, KVH, P], bf16, tag="ktT")
                nc.scalar.activation(ktT, ktp, AF.Copy, bias=-C16, scale=1.0)

                # scores^T [s, kvh, qg] = nib_k^T.T @ q^T (integer-exact)
                scp = ps_sc.tile([P, KVH, QG], f32, tag="scp")
                for h in range(KVH):
                    nc.tensor.matmul(scp[:, h], ktT[:, h], qTs[h],
                                     start=True, stop=True, skip_group_check=True)
                if last:
                    mask_ap = _view(maskb[:], [[0, KVH], [1, QG]])
                    nc.vector.tensor_tensor(scp, scp, mask_ap, op=AL.add)

                # attn[s, qg, kvh] = exp(scores * scale_k/sqrt(D)): per-head
                attn = work.tile([P, QG, KVH], bf16, tag="attn")
                for h in range(KVH):
                    out_ap = _view(attn[:], [[KVH, QG]])
                    out_ap = bass.AP(tensor=out_ap.tensor,
                                     offset=out_ap.offset + h,
                                     ap=out_ap.ap)
                    nc.scalar.activation(out_ap, scp[:, h], AF.Exp,
                                         bias=0.0, scale=psck[:, h:h + 1])

                # attn *= scale_v (all-bf16 packed -> 2x mode)
                svb_b = _view(svb[:], [[0, QG], [1, KVH]])
                nc.vector.tensor_tensor(attn, attn, svb_b, op=AL.mult)

                # denominator: sums[i, j, qg] += sum_s rvb[s, i]*attn[s, qg, j]
                attn_hmaj = _view(attn[:], [[1, KVH], [KVH, QG]])
                nc.tensor.matmul(sums_ps, rvb, attn_hmaj, start=False, stop=last,
                                 skip_group_check=True)
                # numerator: av[d, h, qg] += nib_v[s, h, :]^T @ attn[s, :, h]
                for h in range(KVH):
                    a_ap = _view(attn[:], [[KVH, QG]])
                    a_ap = bass.AP(tensor=a_ap.tensor, offset=a_ap.offset + h,
                                   ap=a_ap.ap)
                    nc.tensor.matmul(av_ps[:, h], nv[:, h], a_ap,
                                     start=False, stop=last, skip_group_check=True)

            # ---- epilogue: normalize, transpose back, store ------------
            # denominators: diag-extract sums^T, all 8 heads into one
            # [QG, KVH] tile, one reciprocal
            sums_sb = fin.tile([KVH, KVH, QG], f32, tag="sums_sb")
            nc.vector.tensor_copy(sums_sb, sums_ps)
            rsall = fin.tile([QG, KVH], f32, tag="rsall")
            for c in range(KVH // 2):
                ch_ps = ps_fin.tile([2 * QG, KVH], f32, tag="pf")
                chunk = sums_sb[:, 2 * c:2 * c + 2].rearrange("h a q -> h (a q)")
                nc.tensor.transpose(ch_ps, chunk, ident_f32[0:KVH, 0:KVH])
                nc.vector.tensor_copy(rsall[:, 2 * c:2 * c + 1],
                                      ch_ps[0:QG, 2 * c:2 * c + 1])
                nc.vector.tensor_copy(rsall[:, 2 * c + 1:2 * c + 2],
                                      ch_ps[QG:2 * QG, 2 * c + 1:2 * c + 2])
            rall = fin.tile([QG, KVH], f32, tag="rall")
            nc.vector.reciprocal(rall, rsall)

            # numerators: batched PSUM->SBUF copy, transpose 4 heads per
            # PSUM bank, one broadcast-normalize + one store per group
            avs_all = fin.tile([D, KVH, QG], f32, tag="avs_all")
            nc.vector.tensor_copy(avs_all, av_ps)
            for grp in range(2):
                ot_ps = ps_fin.tile([QG, 4, D], f32, tag="pf")
                for j in range(4):
                    nc.tensor.transpose(ot_ps[:, j], avs_all[:, 4 * grp + j],
                                        ident_f32)
                ob = fin.tile([QG, 4, D], f32, tag="ob")
                r_view = _view(rall[:, 4 * grp:4 * grp + 4], [[1, 4], [0, D]])
                nc.vector.tensor_tensor(ob, ot_ps, r_view, op=AL.mult)
                for j in range(4):
                    h = 4 * grp + j
                    nc.sync.dma_start(out=o_d[:, G * h:G * (h + 1), :],
                                      in_=ob[:, j])

    if legalize:
        _legalize_waits(nc, mybir)
    return nc


def _legalize_waits(nc, mybir, max_waits=1):
    """walrus codegen has very few sync-wait slots per instruction struct
    (DMA/gpsimd ops fail with >1). Move excess waits onto injected InstNoOp
    pseudo-instructions on the same engine."""
    n = 0
    for blk in nc.m.functions[0].blocks:
        out = []
        for inst in blk.instructions:
            si = inst.sync_info
            if (si is not None and len(si.on_wait) > max_waits
                    and not isinstance(inst, mybir.InstNoOp)):
                waits = list(si.on_wait)
                for w in waits[:-max_waits]:
                    out.append(mybir.InstNoOp(
                        name=f"{inst.name}-wsplit{n}",
                        engine=inst.engine,
                        bass_nofuse=True,
                        sync_info=mybir.SyncInfo(on_wait=[w], on_update=[]),
                    ))
                    n += 1
                inst.sync_info = mybir.SyncInfo(
                    on_wait=waits[-max_waits:], on_update=list(si.on_update))
            out.append(inst)
        blk.instructions = out


def get_nc(reps=1, legalize=True):
    key = f"nc{reps}_{legalize}"
    if key not in _CACHE:
        _CACHE[key] = _build_nc(reps, legalize)
    return _CACHE[key]


def host_mask():
    """[P, QG] f32: -1e30 where last-block position p is masked for query q
    (p >= 113 + q), col = q*G + g."""
    p = np.arange(P)[:, None]
    qq = np.arange(QG)[None, :] // G
    return np.where(p >= 113 + qq, np.float32(-1e30), np.float32(0.0)).astype(np.float32)


def make_in_maps(q, k, v):
    """Host-side sharding/prepacking: per core b, interleave k/v into
    [S, 2, KVH, D] and pre-transpose q into [D, KVH, QG]."""
    q = np.asarray(q, dtype=np.float32)
    k = np.asarray(k, dtype=np.float32)
    v = np.asarray(v, dtype=np.float32)
    maps = []
    for b in range(N_CORES):
        kv = np.ascontiguousarray(np.stack([k[b], v[b]], axis=1))
        qT = np.ascontiguousarray(
            q[b].reshape(Q, KVH, G, D).transpose(1, 3, 0, 2).reshape(KVH, D, QG)
            .transpose(1, 0, 2))
        maps.append({"qT": qT, "kv": kv, "maskc": host_mask()})
    return maps


def kernel(q, k, v, block_table=None, **_unused):
    """Full-input entry point: q [8,16,32,128], k/v [8,4096,8,128] fp32,
    block_table [8,256] int32 (identity permutation). Returns [8,16,32,128]."""
    from concourse.bass_utils import run_bass_kernel_spmd

    nc = get_nc()
    in_maps = make_in_maps(q, k, v)
    res = run_bass_kernel_spmd(nc, in_maps, core_ids=list(range(N_CORES)))
    out = np.stack([np.asarray(res.results[b]["out"]) for b in range(N_CORES)])
    return out.astype(np.float32).reshape(B, Q, H, D)


# revision 24
# speedup vs baseline: 1.0186x; 1.0186x over previous
"""ECC paged-attention kernel for 8x TRN2 NeuronCores (v2).

Semantics (from the reference): the Hamming(8,4) encode/decode round-trip is
exact and the block-table scatter/gather is the identity for any permutation
table (the graded table is arange), so the computation reduces to:

  k_d = round(k/scale_k)*scale_k   (int4 symmetric quant-dequant, per (s, kvh))
  v_d = round(v/scale_v)*scale_v
  out = causal-GQA-attention(q, k_d, v_d)   (queries = last 16 of 4096 positions)

Sharding: batch (8 sequences) across the 8 cores, pure SPMD, no collectives.

v2 changes vs the 163.6us baseline (cost model TimelineSim):
  - pass1 rounding runs in fp16 (t = x*(1/scale) + 1536; fp16 RNE rounds to
    the integer grid exactly like the fp32 + 1.5*2^23 trick, at half the
    dtype width). pass2 (t - 1536 -> bf16) is then an all-16-bit DVE op in
    4x mode, and the k transpose runs at 1 PE cycle/row instead of 2.
  - pass1 slices run mostly on GPSIMD (12), 2 on DVE (2x_2p mode), 2 on ACT;
    the DVE absmax reduce (the old bottleneck's biggest item, no fast mode
    exists for TensorReduce) is issued as ONE batched op over k and v.
  - scale_k is folded into per-head Exp activations (scale AP), removing the
    full-width scores*scale_k DVE multiply.
  - attn is stored [s, qg, kvh] so the scale_v broadcast multiply is an
    all-bf16 packed tensor_tensor (2x_1p); matmuls read strided views.

Engine budget per 128-token block (ns, cost model): DVE ~3.4k (absmax 2194,
pass2-v 327, attn*scale_v 327, smalls ~330, 2 pass1 slices), ACT ~3.5k
(8 fused Exp 1907, ktT -1536 copy 1038, 2 pass1), Pool ~3.3k (12 pass1),
PE ~1.3k, DMA 2912 (the 94.8us HBM floor).
"""

import numpy as np

B, Q, S, H, KVH, D = 8, 16, 4096, 32, 8, 128
G = H // KVH          # 4
QG = Q * G            # 64 rows per kv head
P = 128               # partitions / block size in s
NBLK = S // P         # 32
N_CORES = 8

C16 = 1536.0          # 1.5 * 2**10: fp16 add => round-to-nearest-even integer
INV7 = 1.0 / 7.0
EPS = 1e-8
INV_SQRT_D = 1.0 / float(np.sqrt(D))

_CACHE = {}


def _view(ap, dims):
    """Re-dim a tile AP: keep partition dim, replace free dims with
    [stride, count] pairs."""
    import concourse.bass as bass

    return bass.AP(tensor=ap.tensor, offset=ap.offset, ap=[ap.ap[0]] + dims)


def _build_nc(reps=1, legalize=True):
    from contextlib import ExitStack

    import concourse.bass as bass
    import concourse.tile as tile
    from concourse import mybir
    from concourse.masks import make_identity

    f32 = mybir.dt.float32
    f16 = mybir.dt.float16
    bf16 = mybir.dt.bfloat16
    AL = mybir.AluOpType
    AF = mybir.ActivationFunctionType
    AX = mybir.AxisListType

    nc = bass.Bass("TRN2", target_bir_lowering=False, debug=False,
                   num_devices=N_CORES)

    qt_d = nc.dram_tensor("qT", [D, KVH, QG], f32, kind="ExternalInput").ap()
    kv_d = nc.dram_tensor("kv", [S, 2, KVH, D], f32, kind="ExternalInput").ap()
    o_d = nc.dram_tensor("out", [Q, H, D], f32, kind="ExternalOutput").ap()
    m_d = nc.dram_tensor("maskc", [P, QG], f32, kind="ExternalInput").ap()

    with ExitStack() as ctx:
        tc = ctx.enter_context(tile.TileContext(nc))

        import os
        BUFS = dict(io=6, work=2, stat=5, fin=4, ps_kt=1, ps_sc=3, ps_fin=2, look=3)
        for kv_ in os.environ.get("KBUFS", "").split(","):
            if kv_:
                kk, vv = kv_.split("=")
                BUFS[kk] = int(vv)
        consts = ctx.enter_context(tc.tile_pool(name="consts", bufs=1))
        io = ctx.enter_context(tc.tile_pool(name="io", bufs=BUFS["io"]))
        work = ctx.enter_context(tc.tile_pool(name="work", bufs=BUFS["work"]))
        stat = ctx.enter_context(tc.tile_pool(name="stat", bufs=BUFS["stat"]))
        fin = ctx.enter_context(tc.tile_pool(name="fin", bufs=BUFS["fin"]))
        ps_kt = ctx.enter_context(tc.tile_pool(name="ps_kt", bufs=BUFS["ps_kt"], space="PSUM"))
        ps_sc = ctx.enter_context(tc.tile_pool(name="ps_sc", bufs=BUFS["ps_sc"], space="PSUM"))
        ps_acc = ctx.enter_context(tc.tile_pool(name="ps_acc", bufs=1, space="PSUM"))
        ps_fin = ctx.enter_context(tc.tile_pool(name="ps_fin", bufs=BUFS["ps_fin"], space="PSUM"))

        # ---- early loads: first kv blocks go ahead of everything so the
        # in-order HWDGE queue doesn't stall block 0 behind prologue DMAs
        LOOK = BUFS.get("look", 3)
        kv_tiles = {}

        def load_kv(blk):
            """One DMA per block: kv interleaved [P, 2(kv), KVH, D]."""
            s0 = blk * P
            kvt = io.tile([P, 2, KVH, D], f32, tag="kv")
            nc.sync.dma_start(out=kvt, in_=kv_d[s0:s0 + P])
            kv_tiles[blk] = kvt

        for b0 in range(LOOK):
            load_kv(b0)

        # ---- constants -------------------------------------------------
        ident_f16 = consts.tile([P, P], f16, tag="ident_f16")
        make_identity(nc, ident_f16)
        ident_f32 = consts.tile([P, P], f32, tag="ident_f32")
        make_identity(nc, ident_f32)
        ident_bf = consts.tile([P, P], bf16, tag="ident_bf")
        make_identity(nc, ident_bf)

        # causal-mask bias for the last s-block, [s_in_blk, qg]:
        maskb = consts.tile([P, QG], f32, tag="maskb")
        nc.sync.dma_start(out=maskb, in_=m_d)

        # ---- q prep: host-prepacked q^T, one DMA + one bf16 copy -------
        qtf = consts.tile([D, KVH, QG], f32, tag="qtf")
        nc.sync.dma_start(out=qtf, in_=qt_d)
        qTall = consts.tile([D, KVH, QG], bf16, tag="qTall")
        nc.vector.tensor_copy(qTall, qtf)
        qTs = [qTall[:, h] for h in range(KVH)]

        for _rep in range(reps):
            # ---- persistent accumulators -------------------------------
            av_ps = ps_acc.tile([D, KVH, QG], f32, tag="av")        # attn @ v (^T)
            sums_ps = ps_acc.tile([KVH, KVH, QG], f32, tag="sums")  # denom (diag)
            nc.vector.memset(av_ps, 0.0)
            nc.vector.memset(sums_ps, 0.0)

            # ---- main loop over 128-token blocks -----------------------
            for blk in range(NBLK):
                last = blk == NBLK - 1
                s0 = blk * P

                if blk + LOOK < NBLK:
                    load_kv(blk + LOOK)
                kv = kv_tiles.pop(blk)

                # batched abs-max over D -> [P, 2, KVH] (ONE DVE op)
                am = stat.tile([P, 2, KVH], f32, tag="am")
                nc.vector.tensor_reduce(am, kv, axis=AX.X, op=AL.max,
                                        apply_absolute_value=True)
                sc = stat.tile([P, 2, KVH], f32, tag="sc")
                nc.vector.tensor_scalar(sc, am, INV7, EPS, op0=AL.mult, op1=AL.max)
                rc = stat.tile([P, 2, KVH], f32, tag="rc")
                nc.vector.reciprocal(rc, sc)
                psck = stat.tile([P, KVH], f32, tag="psck")
                nc.vector.tensor_scalar(psck, sc[:, 0], INV_SQRT_D, None,
                                        op0=AL.mult)
                svb = stat.tile([P, KVH], bf16, tag="svb")
                nc.gpsimd.tensor_copy(svb, sc[:, 1])
                rvb = stat.tile([P, KVH], bf16, tag="rvb")
                nc.gpsimd.tensor_copy(rvb, rc[:, 1])

                # pass1: t = x*(1/scale) + 1536 -> fp16 (RNE to integer grid)
                t = work.tile([P, 2, KVH, D], f16, tag="t")
                for c, h in [(0, 0), (0, 1)]:
                    nc.vector.tensor_scalar(t[:, c, h], kv[:, c, h],
                                            rc[:, c, h:h + 1], C16,
                                            op0=AL.mult, op1=AL.add)
                for c, h in [(0, 2), (0, 3)]:
                    nc.scalar.activation(t[:, c, h], kv[:, c, h],
                                         AF.Copy, bias=C16, scale=rc[:, c, h:h + 1])
                for c, h in ([(0, h) for h in range(4, KVH)]
                             + [(1, h) for h in range(KVH)]):
                    nc.gpsimd.tensor_scalar(t[:, c, h], kv[:, c, h],
                                            rc[:, c, h:h + 1], C16,
                                            op0=AL.mult, op1=AL.add)

                # pass2 (v only): nib_v = t_v - 1536 -> bf16 (4x DVE)
                nv = work.tile([P, KVH, D], bf16, tag="nv")
                nc.vector.tensor_scalar(nv, t[:, 1], -C16, None, op0=AL.add)

                # k: transpose t_k (fp16, 1 cyc/row) on PE into ONE psum tile,
                # then one fused ACT pass does (t^T - 1536) -> bf16
                ktp = ps_kt.tile([D, KVH, P], f16, tag="ktp")
                for h in range(KVH):
                    nc.tensor.transpose(ktp[:, h], t[:, 0, h], ident_f16)
                ktT = work.tile([D, KVH, P], bf16, tag="ktT")
                nc.scalar.activation(ktT, ktp, AF.Copy, bias=-C16, scale=1.0)

                # scores^T [s, kvh, qg] = nib_k^T.T @ q^T (integer-exact)
                scp = ps_sc.tile([P, KVH, QG], f32, tag="scp")
                for h in range(KVH):
                    nc.tensor.matmul(scp[:, h], ktT[:, h], qTs[h],
                                     start=True, stop=True, skip_group_check=True)
                if last:
                    mask_ap = _view(maskb[:], [[0, KVH], [1, QG]])
                    nc.vector.tensor_tensor(scp, scp, mask_ap, op=AL.add)

                # attn[s, qg, kvh] = exp(scores * scale_k/sqrt(D)): per-head
                attn = work.tile([P, QG, KVH], bf16, tag="attn")
                for h in range(KVH):
                    out_ap = _view(attn[:], [[KVH, QG]])
                    out_ap = bass.AP(tensor=out_ap.tensor,
                                     offset=out_ap.offset + h,
                                     ap=out_ap.ap)
                    nc.scalar.activation(out_ap, scp[:, h], AF.Exp,
                                         bias=0.0, scale=psck[:, h:h + 1])

                # attn *= scale_v (all-bf16 packed -> 2x mode)
                svb_b = _view(svb[:], [[0, QG], [1, KVH]])
                nc.vector.tensor_tensor(attn, attn, svb_b, op=AL.mult)

                # denominator: sums[i, j, qg] += sum_s rvb[s, i]*attn[s, qg, j]
                attn_hmaj = _view(attn[:], [[1, KVH], [KVH, QG]])
                nc.tensor.matmul(sums_ps, rvb, attn_hmaj, start=False, stop=last,
                                 skip_group_check=True)
                # numerator: av[d, h, qg] += nib_v[s, h, :]^T @ attn[s, :, h]
                for h in range(KVH):
                    a_ap = _view(attn[:], [[KVH, QG]])
                    a_ap = bass.AP(tensor=a_ap.tensor, offset=a_ap.offset + h,
                                   ap=a_ap.ap)
                    nc.tensor.matmul(av_ps[:, h], nv[:, h], a_ap,
                                     start=False, stop=last, skip_group_check=True)

            # ---- epilogue: normalize, transpose back, store ------------
            # denominators: diag-extract sums^T, all 8 heads into one
            # [QG, KVH] tile, one reciprocal
            sums_sb = fin.tile([KVH, KVH, QG], f32, tag="sums_sb")
            nc.vector.tensor_copy(sums_sb, sums_ps)
            rsall = fin.tile([QG, KVH], f32, tag="rsall")
            for c in range(KVH // 2):
                ch_ps = ps_fin.tile([2 * QG, KVH], f32, tag="pf")
                chunk = sums_sb[:, 2 * c:2 * c + 2].rearrange("h a q -> h (a q)")
                nc.tensor.transpose(ch_ps, chunk, ident_f32[0:KVH, 0:KVH])
                nc.vector.tensor_copy(rsall[:, 2 * c:2 * c + 1],
                                      ch_ps[0:QG, 2 * c:2 * c + 1])
                nc.vector.tensor_copy(rsall[:, 2 * c + 1:2 * c + 2],
                                      ch_ps[QG:2 * QG, 2 * c + 1:2 * c + 2])
            rall = fin.tile([QG, KVH], f32, tag="rall")
            nc.vector.reciprocal(rall, rsall)

            # numerators: batched PSUM->SBUF copy, transpose 4 heads per
            # PSUM bank, one broadcast-normalize + one store per group
            avs_all = fin.tile([D, KVH, QG], f32, tag="avs_all")
            nc.vector.tensor_copy(avs_all, av_ps)
            for grp in range(2):
                ot_ps = ps_fin.tile([QG, 4, D], f32, tag="pf")
                for j in range(4):
                    nc.tensor.transpose(ot_ps[:, j], avs_all[:, 4 * grp + j],
                                        ident_f32)
                ob = fin.tile([QG, 4, D], f32, tag="ob")
                r_view = _view(rall[:, 4 * grp:4 * grp + 4], [[1, 4], [0, D]])
                nc.vector.tensor_tensor(ob, ot_ps, r_view, op=AL.mult)
                for j in range(4):
                    h = 4 * grp + j
                    nc.sync.dma_start(out=o_d[:, G * h:G * (h + 1), :],
                                      in_=ob[:, j])

    if legalize:
        _legalize_waits(nc, mybir)
    return nc


def _legalize_waits(nc, mybir, max_waits=1):
    """walrus codegen has very few sync-wait slots per instruction struct
    (DMA/gpsimd ops fail with >1). Move excess waits onto injected InstNoOp
    pseudo-instructions on the same engine."""
    n = 0
    for blk in nc.m.functions[0].blocks:
        out = []
        for inst in blk.instructions:
            si = inst.sync_info
            if (si is not None and len(si.on_wait) > max_waits
                    and not isinstance(inst, mybir.InstNoOp)):
                waits = list(si.on_wait)
                for w in waits[:-max_waits]:
                    out.append(mybir.InstNoOp(
                        name=f"{inst.name}-wsplit{n}",
                        engine=inst.engine,
                        bass_nofuse=True,
                        sync_info=mybir.SyncInfo(on_wait=[w], on_update=[]),
                    ))
                    n += 1
                inst.sync_info = mybir.SyncInfo(
                    on_wait=waits[-max_waits:], on_update=list(si.on_update))
            out.append(inst)
        blk.instructions = out


def get_nc(reps=1, legalize=True):
    key = f"nc{reps}_{legalize}"
    if key not in _CACHE:
        _CACHE[key] = _build_nc(reps, legalize)
    return _CACHE[key]


def host_mask():
    """[P, QG] f32: -1e30 where last-block position p is masked for query q
    (p >= 113 + q), col = q*G + g."""
    p = np.arange(P)[:, None]
    qq = np.arange(QG)[None, :] // G
    return np.where(p >= 113 + qq, np.float32(-1e30), np.float32(0.0)).astype(np.float32)


def make_in_maps(q, k, v):
    """Host-side sharding/prepacking: per core b, interleave k/v into
    [S, 2, KVH, D] and pre-transpose q into [D, KVH, QG]."""
    q = np.asarray(q, dtype=np.float32)
    k = np.asarray(k, dtype=np.float32)
    v = np.asarray(v, dtype=np.float32)
    maps = []
    for b in range(N_CORES):
        kv = np.ascontiguousarray(np.stack([k[b], v[b]], axis=1))
        qT = np.ascontiguousarray(
            q[b].reshape(Q, KVH, G, D).transpose(1, 3, 0, 2).reshape(KVH, D, QG)
            .transpose(1, 0, 2))
        maps.append({"qT": qT, "kv": kv, "maskc": host_mask()})
    return maps


def kernel(q, k, v, block_table=None, **_unused):
    """Full-input entry point: q [8,16,32,128], k/v [8,4096,8,128] fp32,
    block_table [8,256] int32 (identity permutation). Returns [8,16,32,128]."""
    from concourse.bass_utils import run_bass_kernel_spmd

    nc = get_nc()
    in_maps = make_in_maps(q, k, v)
    res = run_bass_kernel_spmd(nc, in_maps, core_ids=list(range(N_CORES)))
    out = np.stack([np.asarray(res.results[b]["out"]) for b in range(N_CORES)])
    return out.astype(np.float32).reshape(B, Q, H, D)
